# revision 54
# baseline (speedup 1.0000x reference)
"""Trainium2 Bass kernel for nn_BEMBFlex (within-category log-softmax utility).

Sharding: items dealt by category across 8 cores (categories rank-sorted by
size, rank % 8 -> shard), one SPMD program for all cores. Each core computes
util for all 1024 sessions over its ~1/8 of items, then the within-category
log-softmax locally.

Layout (the key trick vs the previous version): within each block, item
columns are POSITION-MAJOR: col = t * g + q for slot q < g, within-category
position t < L. Consequences:
  - segment sums become a cascade of CONTIGUOUS bf16 adds (DVE 2x mode),
  - the final log-prob op is an int16 TT subtract whose lsc operand is a
    4D AP [part, [256,8], [0,L'], [1,g]] with stride-1 last dim -> 2x mode
    (the old slot-major broadcast AP had a stride-0 last dim -> stuck at 1x),
  - all 8 session-chunks of a block are processed by ONE DVE op via a
    chunk-stride AP dim, amortizing instruction overhead 8x.

Output is in log-bits scale: out_i16 = bits16(ex) - bits16(s), where
ln(x) ~ bits16(x)*ln2/128 - 127*ln2 (biases cancel in the subtraction).
Host multiplies by K16 = ln2/128 during the de-permute.

Blocks are sized g*L <= 512 so one PSUM bank-aligned 512-region holds one
(block, chunk) pair; a [P, 2048] PSUM tile holds 4 chunks and is drained by
a single gap-skipping 3D-AP ScalarE Exp. Per region the PE streams the util
matmul plus a rank-1 lambda matmul (fp8 hi+lo DoubleRow planes; exact to
~2^-7). For ND_BLOCKS "D-blocks" the lambda matmul is skipped to offload
the (p-state-capped) PE: DVE multiplies ex0 by a partition-replicated
exp(lambda) row (one 2x TT op per block, zeros doubling as the pad mask)
and the host adds lambda back to those columns exactly.

Measured notes: the PE p-state never reaches full speed once any
cross-engine semaphore wait appears in its queue (every PSUM-ring tile),
so matmuls run at the mid p-state 0.833 ns/col; DVE 2x_1p requires every
non-scalar operand packed stride-1 in its last AP dim (the pos-major
layout exists to satisfy this); GpSimd supports no tensor-tensor/STT ops;
DMA issue costs ~0.6-0.8us of queue time per dma_start on every ring, so
bulk data moves as few multi-dim DMAs on the otherwise-idle gpsimd queue.
"""

import sys

for _p in ("/opt/trn_rl_repo",):
    if _p not in sys.path:
        sys.path.insert(0, _p)

import ml_dtypes
import numpy as np

import concourse.bass as bass
import concourse.tile as tile
from concourse import bacc, bass_utils, mybir

NUM_USERS = 100000
NUM_ITEMS = 25000
NUM_CATS = 500
LATENT = 64
BATCH = 1024
NCORES = 8
P = 128
NCHUNKS = BATCH // P
REGION = 512          # psum bank-aligned region = one (block, chunk)
CSTR_EX = 512         # ex tile per-chunk stride
CSTR_TR = 256         # tree scratch per-chunk stride
PAD8 = -224.0         # fp8e4m3 pad: hi+lo = -448 -> exp underflows to 0
ND_BLOCKS = 2         # blocks whose lambda runs on DVE instead of the PE
LN2 = float(np.log(2.0))
K16 = LN2 / (1 << 7)  # bf16-bit -> ln scale

F32 = mybir.dt.float32
BF16 = mybir.dt.bfloat16
I16 = mybir.dt.int16
FP8 = mybir.dt.float8e4

_nc_cache = {}


# ----------------------------------------------------------------------------
# Host-side layout
# ----------------------------------------------------------------------------

def _layout(cat_sizes):
    """Blocks of slots with uniform tree width L, g*L <= REGION.

    Categories sorted by size desc; slot i holds ranks [8i, 8i+8) (one per
    shard). L = max size in the leading slot rounded up to even; g = how many
    slots fit in a 512 region. Lp = max true size in the block (final/out
    cover only positions t < Lp).
    """
    order = np.argsort(-cat_sizes, kind="stable")
    order = order[cat_sizes[order] > 0]
    ncats = len(order)
    nslots = -(-ncats // NCORES)
    slot_max = np.empty(nslots, np.int64)
    for i in range(nslots):
        slot_max[i] = int(cat_sizes[order[i * NCORES]])
    blocks = []  # (slot0, g, L, Lp)
    i = 0
    while i < nslots:
        Lp = int(slot_max[i])
        L = Lp + (Lp & 1)
        L = max(L, 2)
        g = min(REGION // L, nslots - i)
        blocks.append((i, g, L, Lp))
        i += g
    return order, blocks


def _prep(inputs):
    cat = np.asarray(inputs["category_idx"]).astype(np.int64).ravel()
    cat_sizes = np.bincount(cat, minlength=NUM_CATS)
    order, blocks = _layout(cat_sizes)
    nb = len(blocks)

    rank = np.full(NUM_CATS, -1, np.int64)
    rank[order] = np.arange(len(order))

    perm = np.argsort(cat, kind="stable")
    starts = np.searchsorted(cat[perm], np.arange(NUM_CATS))
    within_sorted = np.arange(NUM_ITEMS) - starts[cat[perm]]
    item_within = np.empty(NUM_ITEMS, np.int64)
    item_within[perm] = within_sorted

    # per-slot -> block index, q, and per-block col bases
    nslots = -(-len(order) // NCORES)
    blk_of_slot = np.empty(nslots, np.int64)
    q_of_slot = np.empty(nslots, np.int64)
    g_of_slot = np.empty(nslots, np.int64)
    ob0 = np.empty(nb, np.int64)
    acc = 0
    for b, (s0, g, L, Lp) in enumerate(blocks):
        blk_of_slot[s0:s0 + g] = b
        q_of_slot[s0:s0 + g] = np.arange(g)
        g_of_slot[s0:s0 + g] = g
        ob0[b] = acc
        acc += g * Lp
    opad = acc

    r = rank[cat]
    slot = r // NCORES
    item_shard = r % NCORES
    blk = blk_of_slot[slot]
    item_wcol = blk * REGION + item_within * g_of_slot[slot] + q_of_slot[slot]
    item_ocol = ob0[blk] + item_within * g_of_slot[slot] + q_of_slot[slot]

    alpha = np.ascontiguousarray(np.asarray(inputs["alpha_item"], np.float32))
    obs = np.ascontiguousarray(np.asarray(inputs["item_obs"], np.float32))
    lam = np.asarray(inputs["lambda_item"], np.float32).ravel()

    # D-blocks: lambda applied on DVE as ex0 * exp(lam) with a replicated
    # row (PE rank-1 matmul skipped); host adds lam back to their outputs.
    sz = [g * L for (_s, g, L, _p) in blocks]
    order_blocks = [0] + sorted(range(1, nb), key=lambda b: -sz[b])
    dblocks = tuple(sorted(order_blocks[:ND_BLOCKS]))

    wpad = nb * REGION
    W = np.zeros((NCORES, 2 * LATENT, wpad), np.float32)
    # lambda as fp8e4m3 hi+lo planes for the DoubleRow rank-1 matmul:
    # block b occupies [b*1024, b*1024+512) = hi, [+512, +1024) = lo
    LAMS8 = np.full((NCORES, 1, nb * 2 * REGION), PAD8, np.float32)
    for s in range(NCORES):
        m = item_shard == s
        cols = item_wcol[m]
        W[s, 0:LATENT, cols] = alpha[m]
        W[s, LATENT:, cols] = obs[m]
        blk_c = cols // REGION
        off_c = cols % REGION
        hi = np.asarray(lam[m].astype(ml_dtypes.float8_e4m3fn), np.float32)
        lo = lam[m] - hi
        LAMS8[s, 0, blk_c * 2 * REGION + off_c] = hi
        LAMS8[s, 0, blk_c * 2 * REGION + REGION + off_c] = lo
    W = W.astype(ml_dtypes.bfloat16)
    LAMS8 = LAMS8.astype(ml_dtypes.float8_e4m3fn)

    # replicated exp(lam) rows for D-blocks (zeros double as the pad mask)
    dmap = np.full(nb, -1, np.int64)
    for i, b_ in enumerate(dblocks):
        dmap[b_] = i
    ELAMR = np.zeros((NCORES, len(dblocks) * REGION), np.float32)
    lamadd = np.zeros(NUM_ITEMS, np.float32)
    for s in range(NCORES):
        m = item_shard == s
        cols = item_wcol[m]
        bc = cols // REGION
        sel = dmap[bc] >= 0
        dcols = dmap[bc[sel]] * REGION + cols[sel] % REGION
        ELAMR[s, dcols] = np.exp(lam[m][sel])
    in_d = dmap[item_wcol // REGION] >= 0
    lamadd[in_d] = lam[in_d] / K16

    uidx = np.asarray(inputs["user_index"]).astype(np.int64).ravel()
    theta = np.asarray(inputs["theta_user"], np.float32)
    zeta = np.asarray(inputs["zeta_user"], np.float32)
    thzet = np.ascontiguousarray(
        np.concatenate([theta[uidx], zeta[uidx]], axis=1).T
    ).astype(ml_dtypes.bfloat16)
    return {
        "blocks": blocks,
        "opad": opad,
        "item_shard": item_shard,
        "item_ocol": item_ocol,
        "W": W,
        "LAMS8": LAMS8,
        "ELAMR": ELAMR,
        "lamadd": lamadd,
        "thzet": thzet,
    }


# ----------------------------------------------------------------------------
# Device program
# ----------------------------------------------------------------------------

def _ap3(t2d, off, cstr, n, w):
    """[P, N] tile -> [P, n, w] AP: chunk-stride cstr, packed inner width."""
    ap = t2d[:, :]
    return bass.AP(tensor=ap.tensor, offset=ap.offset + off,
                   ap=[ap.ap[0], [cstr, n], [1, w]])


def _ap4(t2d, off, cstr, n, rep, w):
    """[P, N] tile -> [P, n, rep, w] AP with a step-0 middle dim."""
    ap = t2d[:, :]
    return bass.AP(tensor=ap.tensor, offset=ap.offset + off,
                   ap=[ap.ap[0], [cstr, n], [0, rep], [1, w]])


def _build_nc(blocks, opad):
    nb = len(blocks)
    wpad = nb * REGION
    nc = bacc.Bacc(
        "TRN2",
        debug=False,
        enable_asserts=False,
        target_bir_lowering=False,
        num_devices=NCORES,
        enable_partition_id=False,
    )
    w_d = nc.dram_tensor("W", [2 * LATENT, wpad], BF16, kind="ExternalInput").ap()
    lams_d = nc.dram_tensor("LAMS8", [1, nb * 2 * REGION], FP8,
                            kind="ExternalInput").ap()
    thzet_d = nc.dram_tensor("THZET", [2 * LATENT, BATCH], BF16,
                             kind="ExternalInput").ap()
    out_d = nc.dram_tensor("O", [BATCH, opad], I16, kind="ExternalOutput").ap()

    # process order: block 0 first (its W slice is one small leading DMA),
    # then descending by cols so the drain block is the smallest
    sz = [g * L for (_s, g, L, _p) in blocks]
    order_blocks = [0] + sorted(range(1, nb), key=lambda b: -sz[b])
    dblocks = tuple(sorted(order_blocks[:ND_BLOCKS]))
    dmap = {b: i for i, b in enumerate(dblocks)}
    elamr_d = nc.dram_tensor("ELAMR", [P, len(dblocks) * REGION], BF16,
                             kind="ExternalInput").ap()

    with tile.TileContext(nc) as tc:
        with (
            tc.tile_pool(name="singles", bufs=1) as singles,
            tc.tile_pool(name="psum_u", bufs=2, space="PSUM") as psum_u,
            tc.tile_pool(name="exbuf", bufs=4) as exbuf,
            tc.tile_pool(name="exlbuf", bufs=2) as exlbuf,
            tc.tile_pool(name="treebuf", bufs=4) as treebuf,
            tc.tile_pool(name="obuf", bufs=4) as obuf,
        ):
            thzet_sb = singles.tile([2 * LATENT, BATCH], BF16, name="thzet_sb")
            # first 4 chunks land early so the PE can start sooner
            nc.sync.dma_start(out=thzet_sb[:, 0:4 * P], in_=thzet_d[:, 0:4 * P])
            ones8_sb = singles.tile([1, 2 * P], FP8, name="ones8_sb")
            nc.vector.memset(ones8_sb[:, :], 1.0)
            thze_t = [thzet_sb[:, c * P:(c + 1) * P] for c in range(NCHUNKS)]
            w_sb = singles.tile([2 * LATENT, wpad], BF16, name="w_sb")
            lams_sb = singles.tile([1, nb * 2 * REGION], FP8, name="lams_sb")
            # W block 0 on the sync ring: it issues ~1us before gpsimd's
            # DGE comes up, so the first matmul starts sooner
            nc.sync.dma_start(out=w_sb[:, 0:REGION], in_=w_d[:, 0:REGION])
            nc.sync.dma_start(out=thzet_sb[:, 4 * P:], in_=thzet_d[:, 4 * P:])
            elamr_sb = singles.tile([P, len(dblocks) * REGION], BF16,
                                    name="elamr_sb")
            nc.gpsimd.dma_start(out=elamr_sb[:, :], in_=elamr_d[:, :])
            nc.gpsimd.dma_start(out=lams_sb[:, :], in_=lams_d[:, :])
            nc.gpsimd.dma_start(out=w_sb[:, REGION:], in_=w_d[:, REGION:])
            # lhsT for the DoubleRow rank-1: [K=1, ktile=2, M=128] of ones
            ones8_ap = bass.AP(
                tensor=ones8_sb[:, :].tensor, offset=ones8_sb[:, :].offset,
                ap=[ones8_sb[:, :].ap[0], [P, 2], [1, P]],
            )

            ob0s = []
            acc = 0
            for (s0, g, L, Lp) in blocks:
                ob0s.append(acc)
                acc += g * Lp
            split_bs = set(order_blocks[-3:]) | {order_blocks[0]}
            for b in order_blocks:
                (s0, g, L, Lp) = blocks[b]
                ob0 = ob0s[b]
                wc0 = b * REGION
                cols = g * L
                ex = exbuf.tile([P, NCHUNKS * CSTR_EX], BF16, name="ex", tag="ex")
                # PE + ScalarE: two tiles of 4 chunk-regions each
                is_d = b in dmap
                lam_rhs_base = lams_sb[:, :]
                for half in range(2):
                    up = psum_u.tile([P, 4 * REGION], F32, name="up", tag="up")
                    for ci in range(4):
                        c = half * 4 + ci
                        nc.tensor.matmul(
                            up[:, ci * REGION:ci * REGION + cols],
                            lhsT=thze_t[c],
                            rhs=w_sb[:, wc0:wc0 + cols],
                            start=True, stop=is_d,
                        )
                    if not is_d:
                        for ci in range(4):
                            # rank-1 lambda: fp8 hi+lo DoubleRow
                            lam_rhs = bass.AP(
                                tensor=lam_rhs_base.tensor,
                                offset=lam_rhs_base.offset + b * 2 * REGION,
                                ap=[lam_rhs_base.ap[0], [REGION, 2], [1, cols]],
                            )
                            nc.tensor.matmul(
                                up[:, ci * REGION:ci * REGION + cols],
                                lhsT=ones8_ap,
                                rhs=lam_rhs,
                                start=False, stop=True,
                                perf_mode=mybir.MatmulPerfMode.DoubleRow,
                            )
                    # 3D APs skip the (512 - g*L) pad gap of each region
                    nc.scalar.activation(
                        out=_ap3(ex, half * 4 * CSTR_EX, CSTR_EX, 4, cols),
                        in_=_ap3(up, 0, REGION, 4, cols),
                        func=mybir.ActivationFunctionType.Exp,
                    )
                # DVE: cascade of contiguous adds, fused across chunk groups;
                # the last processed block splits into halves so its final +
                # out-DMA overlap the second half's exp (shorter drain)
                ob = obuf.tile([P, NCHUNKS * g * Lp], I16, name="ob", tag="ob")
                groups = [(0, 4), (4, 4)] if b in split_bs else [(0, NCHUNKS)]
                exs = ex
                if is_d:
                    exl = exlbuf.tile([P, NCHUNKS * CSTR_EX], BF16,
                                      name="exl", tag="exl")
                    exs = exl
                for (c0, ncr) in groups:
                    if is_d:
                        # lambda on DVE: exl = ex0 * exp(lam) row (2x TT
                        # mult; zeros mask pads). Per-group so the first
                        # half starts right after its EXP.
                        el = elamr_sb[:, :]
                        el3 = bass.AP(tensor=el.tensor,
                                      offset=el.offset + dmap[b] * REGION,
                                      ap=[el.ap[0], [0, ncr], [1, cols]])
                        nc.vector.tensor_tensor(
                            out=_ap3(exl, c0 * CSTR_EX, CSTR_EX, ncr, cols),
                            in0=_ap3(ex, c0 * CSTR_EX, CSTR_EX, ncr, cols),
                            in1=el3,
                            op=mybir.AluOpType.mult,
                        )
                    trA = treebuf.tile([P, NCHUNKS * CSTR_TR], BF16,
                                       name="trA", tag="trA")
                    exo = c0 * CSTR_EX
                    tro = c0 * CSTR_TR
                    h = L // 2
                    nc.vector.tensor_add(
                        out=_ap3(trA, tro, CSTR_TR, ncr, g * h),
                        in0=_ap3(exs, exo, CSTR_EX, ncr, g * h),
                        in1=_ap3(exs, exo + g * h, CSTR_EX, ncr, g * h),
                    )
                    w_ = h
                    while w_ > 1:
                        # in-place halving: out == in0 slab; odd-w middle
                        # column stays untouched and joins the next level
                        hc = (w_ + 1) // 2
                        hh = w_ - hc
                        nc.vector.tensor_add(
                            out=_ap3(trA, tro, CSTR_TR, ncr, g * hh),
                            in0=_ap3(trA, tro, CSTR_TR, ncr, g * hh),
                            in1=_ap3(trA, tro + g * hc, CSTR_TR, ncr, g * hh),
                        )
                        w_ = hc
                    # s (bf16) sits at offset 0 of each chunk segment
                    nc.vector.tensor_tensor(
                        out=_ap3(ob, c0 * g * Lp, g * Lp, ncr, g * Lp),
                        in0=_ap3(ex, exo, CSTR_EX, ncr, g * Lp).bitcast(I16),
                        in1=_ap4(trA, tro, CSTR_TR, ncr, Lp, g).bitcast(I16),
                        op=mybir.AluOpType.subtract,
                    )
                    # one DMA per group: DRAM rows c*128+p <- SBUF cols
                    od = out_d[:, ob0:ob0 + g * Lp]
                    od3 = bass.AP(
                        tensor=od.tensor,
                        offset=od.offset + c0 * P * opad,
                        ap=[[opad, P], [opad * P, ncr], [1, g * Lp]],
                    )
                    nc.gpsimd.dma_start(
                        out=od3,
                        in_=_ap3(ob, c0 * g * Lp, g * Lp, ncr, g * Lp),
                    )
    nc.compile()
    return nc


# ----------------------------------------------------------------------------
# Entry points
# ----------------------------------------------------------------------------

def run(inputs, trace=False):
    prep = _prep(inputs)
    key = (prep["opad"], tuple(prep["blocks"]))
    nc = _nc_cache.get(key)
    if nc is None:
        print(f"[kernel] opad={prep['opad']} nb={len(prep['blocks'])} "
              f"blocks={prep['blocks']}", file=sys.stderr)
        nc = _build_nc(prep["blocks"], prep["opad"])
        _nc_cache[key] = nc
    in_maps = [
        {
            "W": prep["W"][c],
            "LAMS8": prep["LAMS8"][c],
            "ELAMR": np.ascontiguousarray(
                np.broadcast_to(prep["ELAMR"][c][None, :],
                                (P, prep["ELAMR"].shape[1]))
            ).astype(ml_dtypes.bfloat16),
            "THZET": prep["thzet"],
        }
        for c in range(NCORES)
    ]
    res = bass_utils.run_bass_kernel_spmd(
        nc, in_maps, core_ids=list(range(NCORES)), trace=trace
    )
    big = np.stack(
        [np.asarray(res.results[c]["O"]) for c in range(NCORES)]
    )  # [8, B, opad] i16
    out = (
        big[prep["item_shard"], :, prep["item_ocol"]].T
        + prep["lamadd"][None, :]
    ).astype(np.float32) * np.float32(K16)
    return out, res


def kernel(**inputs) -> np.ndarray:
    out, _ = run(inputs, trace=False)
    return out
